# revision 25
# baseline (speedup 1.0000x reference)
"""Trainium2 Bass kernel for nn_Block_34067680592489.

Computes, for B=32768 independent signals x[b] (length 256):
  mu,reg = small-CNN(x[b])      (conv5+avgpool4+softplus twice, linear, softplus)
  grad   = TtT x - x_b + reg * DtD x
  x_t    = x - gamma * grad,  gamma = softplus(gamma_p)
  out    = middle root of z^3 -(m+x_t) z^2 + (m x_t - 2 gm) z + gm m,  gm = gamma*mu

Device algorithm (signal-on-partitions layout, all matmuls fp16):
  s[n,b]  = x W_A + (reg*x) W_B + xb3            (single PSUM accumulation:
            W_A = (I-g TtT^T)/3m, W_B = -g DtD^T/3m, xb3 = g/(3m) x_b + 1/3;
            reg-fold via pre-scaled moving operand, xb3-fold via identity-
            block stationary operand)
  hm   = (s-1/2)^2 + (2/3) gh + 1/4              (DVE fused)
  q    = ((s-1/2)^2 + gh) * (1-2s)               (DVE fused)
  ihm3 = 1/(3 hm)                                 (ACT reciprocal table)
  e    = y'*P(t'), y'=q*ihm3, t'=y'^2*ihm3        (DVE fused; P = deg-2 poly
          fitted so e ~= r*sin(arcsin(w)/3), w = q/(2 hm^1.5), r = 2 sqrt(hm))
  H    = q + e^3                                  (DVE fused; one fixed-point
          refinement of the triple-angle identity 3 sigma - 4 sigma^3 = w)
  out  = s + H * ihm3                             (GpSimd mult + DVE add)
No ln/exp/trig at all -> single ACT table switch (softplus CNN via exp/ln
happens entirely before the reciprocal phase).

Sharding: pure data parallel over batch, 8 cores x 4096 rows.
"""

import numpy as np

B_TOTAL = 32768
N = 256
N_CORES = 8
BC = B_TOTAL // N_CORES      # rows per core (4096)
G = 512                      # rows per group/chunk
NG = BC // G                 # 8 groups

# deg-2 odd-poly fit of sin(arcsin(w)/3)/w on w in [0, 0.825], rescaled for
# the t' = y'^2*ihm3 argument (y' = q*ihm3, ihm3 = 1/(3 hm)).
SIG_C0 = 1.0007408750509472
SIG_C1 = 0.8616695104467121
SIG_C2 = 6.336400405433818

_PROG = {}


def _np_f32(a):
    return np.ascontiguousarray(np.asarray(a, dtype=np.float32))


def _conv_pool_mat(w, L):
    """(L/4, L) matrix implementing conv1d(k=5,pad=2) then avgpool4."""
    taps = np.asarray(w, np.float32).reshape(5)
    C = np.zeros((L, L), np.float32)
    for n in range(L):
        for k in range(5):
            m = n + k - 2
            if 0 <= m < L:
                C[n, m] = taps[k]
    P = np.zeros((L // 4, L), np.float32)
    for i in range(L // 4):
        P[i, 4 * i:4 * i + 4] = 0.25
    return (P @ C).astype(np.float32)


_CUSTOM_OPS = {}


def _get_custom_ops():
    """Register this kernel's fused custom-DVE ops (idempotent)."""
    if _CUSTOM_OPS:
        return _CUSTOM_OPS
    import concourse.dve_ops as dops
    from concourse.dve_spec import (Spec, Src0, Src1, C0, C1, C2, One,
                                    sq, lower, _has_src1)
    from concourse.dve_uop import DveOpSpec

    def reg(name, spec):
        if name in dops._SUB_OPCODE_FOR_NAME:
            return next(o for o in dops.OPS if o.name == name)
        row = dops._CUSTOM_DVE_ROW_BASE + len(dops.OPS)
        assert row < 0x20
        dops._SUB_OPCODE_FOR_NAME[name] = row
        shas = {}
        for ver in ("v3", "v4"):
            u = lower(spec, ver=ver)
            shas[ver] = DveOpSpec(name=name, opcode=row, uops=u,
                                  rd1_en=_has_src1(spec)).sha(ver)
        op = dops.DveOp(name, spec, subdim=False, uops_sha=shas)
        dops.OPS.append(op)
        dops.CUSTOM_DVE_SPECS[name] = spec
        return op

    import numpy as np_

    # hm = (s-imm2)^2 + (gh*s0 + s1)
    _CUSTOM_OPS['HM'] = reg('ANT_K_HM2', Spec(
        body=sq(Src0 - C2) + (Src1 * C0 + C1),
        reference=lambda in0, in1, s0, s1, imm2:
            ((in0 - imm2) ** 2 + (in1 * s0 + s1)).astype(np_.float32),
    ))
    # q = ((s-imm2)^2 + (mu*s0 + s1)) * (1 - 2 s)
    _CUSTOM_OPS['QQ'] = reg('ANT_K_QQ3', Spec(
        body=(sq(Src0 - C2) + (Src1 * C0 + C1)) * (One - (Src0 + Src0)),
        reference=lambda in0, in1, s0, s1, imm2:
            (((in0 - imm2) ** 2 + (in1 * s0 + s1)) * (1.0 - (in0 + in0))
             ).astype(np_.float32),
    ))
    # e = y*(C0 + t*(C1 + t*C2)), y = q*ihm3, t = y^2*ihm3
    _y = Src0 * Src1
    _t = sq(_y) * Src1
    _CUSTOM_OPS['SIGE'] = reg('ANT_K_SIGE', Spec(
        body=_y * (C0 + _t * (C1 + _t * C2)),
        reference=lambda in0, in1, s0, s1, imm2:
            ((in0 * in1) * (s0 + (in0 * in1) ** 2 * in1
                            * (s1 + (in0 * in1) ** 2 * in1 * imm2))
             ).astype(np_.float32),
    ))
    # H = (q + e^3) * s0
    _CUSTOM_OPS['QE3'] = reg('ANT_K_QE3B', Spec(
        body=(Src0 + sq(Src1) * Src1) * C0,
        reference=lambda in0, in1, s0, s1, imm2:
            ((in0 + in1 ** 3) * s0).astype(np_.float32),
    ))
    return _CUSTOM_OPS


_TABLES_PATCHED = False


def _patch_act_tables():
    """Restrict ACT table-set choice to the two sets this kernel uses:
    natural_log_exp (CNN softplus) and reciprocal (cardan 1/(3 hm))."""
    global _TABLES_PATCHED
    if _TABLES_PATCHED:
        return
    import concourse.bacc as bacc
    keep = {'natural_log_exp_and_others'}
    orig = bacc.get_activation_tables

    def patched(arch):
        t = orig(arch)
        return {k: (v if k in keep else set()) for k, v in t.items()}

    bacc.get_activation_tables = patched
    _TABLES_PATCHED = True


def _build_program(gamma, m):
    import concourse.bacc as bacc
    import concourse.tile as tile
    import concourse.mybir as mybir
    _patch_act_tables()

    dt = mybir.dt
    f32 = dt.float32
    f16 = dt.float16
    Alu = mybir.AluOpType
    AF = mybir.ActivationFunctionType

    COPS = _get_custom_ops()
    nc = bacc.Bacc("TRN2", target_bir_lowering=False, debug=False,
                   num_devices=N_CORES)

    XT = nc.dram_tensor("xt", (256, BC), f16, kind="ExternalInput")
    XB3T = nc.dram_tensor("xb3t", (256, BC), f16, kind="ExternalInput")
    WM = nc.dram_tensor("wm", (256, 512), f16, kind="ExternalInput")
    I128 = nc.dram_tensor("i128", (128, 128), f16, kind="ExternalInput")
    M1T = nc.dram_tensor("m1t", (256, 128), f16, kind="ExternalInput")
    M2BD = nc.dram_tensor("m2bd", (128, 32), f16, kind="ExternalInput")
    LWBD = nc.dram_tensor("lwbd", (32, 2), f16, kind="ExternalInput")
    B2V = nc.dram_tensor("b2v", (128, 1), f32, kind="ExternalInput")
    B3V = nc.dram_tensor("b3v", (32, 1), f32, kind="ExternalInput")
    LBMR = nc.dram_tensor("lbmr", (2, 1), f32, kind="ExternalInput")
    OUT = nc.dram_tensor("out", (256, BC), f16, kind="ExternalOutput")

    gm2 = float(gamma / (m * m))

    with tile.TileContext(nc) as tc:
        with (
            tc.tile_pool(name="const", bufs=1) as cpool,
            tc.tile_pool(name="xin", bufs=1) as xpool,
            tc.tile_pool(name="scal", bufs=1) as spool,
            tc.tile_pool(name="cnn", bufs=2) as cnnpool,
            tc.tile_pool(name="hq", bufs=4) as hqpool,
            tc.tile_pool(name="s16", bufs=6) as s16pool,
            tc.tile_pool(name="mid", bufs=3) as midpool,
            tc.tile_pool(name="oout", bufs=5) as opool,
            tc.tile_pool(name="pm", bufs=2, space="PSUM") as pmpool,
            tc.tile_pool(name="rpb", bufs=2, space="PSUM") as rpbpool,
            tc.tile_pool(name="pc1", bufs=1, space="PSUM") as pc1pool,
            tc.tile_pool(name="pc23", bufs=1, space="PSUM") as pc23pool,
        ):
            # ---- tiles ----
            wm = cpool.tile([128, 2, 512], f16)
            i128 = cpool.tile([128, 128], f16)
            m1t = cpool.tile([128, 2, 128], f16)
            m2bd = cpool.tile([128, 32], f16)
            lwbd = cpool.tile([32, 2], f16)
            b2v = cpool.tile([128, 1], f32)
            b3v = cpool.tile([32, 1], f32)
            lbmr = cpool.tile([2, 1], f32)
            xt = xpool.tile([128, 2, BC], f16)
            xb3t = xpool.tile([128, 2, BC], f16)
            xreg = xpool.tile([128, 2, BC], f16)
            sp = spool.tile([2, BC], f16)      # row0 = reg, row1 = mu
            murow = spool.tile([1, BC], f16)
            mub = spool.tile([128, BC], f16)
            ones1 = spool.tile([1, 128], f16)
            nc.vector.memset(ones1[:], 1.0)

            # ---- input DMAs, most-needed first ----
            for k in range(2):
                nc.sync.dma_start(m1t[:, k, :], M1T[128 * k:128 * (k + 1), :])
            for k in range(2):
                nc.sync.dma_start(xt[:, k, 0:BC // 4],
                                  XT[128 * k:128 * (k + 1), 0:BC // 4])
            for k in range(2):
                nc.sync.dma_start(wm[:, k, :], WM[128 * k:128 * (k + 1), :])
            nc.sync.dma_start(i128[:], I128[:])
            nc.sync.dma_start(m2bd[:], M2BD[:])
            nc.sync.dma_start(lwbd[:], LWBD[:])
            nc.sync.dma_start(b2v[:], B2V[:])
            nc.sync.dma_start(b3v[:], B3V[:])
            nc.sync.dma_start(lbmr[:], LBMR[:])
            for qq in range(4):
                qsl = slice(BC // 4 * qq, BC // 4 * (qq + 1))
                for k in range(2):
                    nc.sync.dma_start(xb3t[:, k, qsl],
                                      XB3T[128 * k:128 * (k + 1), qsl])
                if qq > 0:
                    for k in range(2):
                        nc.sync.dma_start(xt[:, k, qsl],
                                          XT[128 * k:128 * (k + 1), qsl])

            hm_t = [None] * NG
            q_t = [None] * NG
            s16_t = [None] * NG
            ihm3_t = [None] * NG
            H_t = [None] * NG
            rg_t = [None] * NG
            ot_t = [None] * NG
            pm_t = [None] * NG

            def csl_of(c):
                return slice(G * c, G * (c + 1))

            # Software-pipelined schedule; chunk c: main matmuls at iter
            # c+1, s evac c+1, hm/q + ln/exp c+2, sigma chain c+3, final +
            # store c+4. Queue order per iteration keeps the critical cycle
            # (CNN ladder -> sp -> broadcast -> xreg -> main matmuls) free
            # of unrelated head-of-line waits on every engine.
            for g in range(NG + 5):
                gsl = slice(G * g, G * (g + 1))

                # PE: CNN level-1 matmul for group g (always ready)
                if g < NG:
                    p1 = pc1pool.tile([128, G], f32, tag="p1",
                                      name=f"p1g{g}")
                    nc.tensor.matmul(p1[:], m1t[:, 0, :], xt[:, 0, gsl],
                                     start=True, stop=False)
                    nc.tensor.matmul(p1[:], m1t[:, 1, :], xt[:, 1, gsl],
                                     start=False, stop=True)

                # PE: broadcast reg row via rank-1 matmul (ones x reg)
                rpb = None
                if 1 <= g <= NG:
                    c = g - 1
                    rpb = rpbpool.tile([128, G], f32, tag="rpb",
                                       name=f"rpb{c}")
                    nc.tensor.matmul(rpb[:], ones1[:], sp[0:1, csl_of(c)],
                                     start=True, stop=True)
                # GpSimd: mu broadcast for g-1 (off critical path)
                if 1 <= g <= NG:
                    c = g - 1
                    nc.gpsimd.partition_broadcast(mub[:, csl_of(c)],
                                                  murow[:, csl_of(c)])

                # GpSimd: rg for chunk g-3 (after final below issues first)
                if 4 <= g < NG + 4:
                    c = g - 4
                    ot = opool.tile([128, 2, G], f16, tag="ot",
                                    name=f"ot{c}")
                    ot_t[c] = ot
                    nc.gpsimd.tensor_tensor(
                        ot[:].rearrange("p a b -> p (a b)"), s16_t[c][:],
                        rg_t[c][:], Alu.add)
                    dview = OUT.rearrange("(h p) r -> p h r",
                                          p=128)[:, :, csl_of(c)]
                    nc.sync.dma_start(dview, ot[:])

                # Vector: xreg for g-1 (critical), then trailing stages
                if 1 <= g <= NG:
                    c = g - 1
                    for k in range(2):
                        nc.vector.tensor_tensor(xreg[:, k, csl_of(c)],
                                                xt[:, k, csl_of(c)],
                                                rpb[:], Alu.mult)

                # PE: main matmuls for chunk g-1; the xreg-independent
                # A/identity parts are issued before the B parts so they
                # overlap the broadcast -> xreg chain.
                if 1 <= g <= NG:
                    c = g - 1
                    csl = csl_of(c)
                    pm = pmpool.tile([128, 2, G], f32, tag="pm",
                                     name=f"pm{c}")
                    pm_t[c] = pm
                    for nb in range(2):
                        nc.tensor.matmul(pm[:, nb, :],
                                         wm[:, 0, 128 * nb:128 * (nb + 1)],
                                         xt[:, 0, csl], start=True,
                                         stop=False)
                        nc.tensor.matmul(pm[:, nb, :],
                                         wm[:, 1, 128 * nb:128 * (nb + 1)],
                                         xt[:, 1, csl], start=False,
                                         stop=False)
                        nc.tensor.matmul(pm[:, nb, :], i128[:],
                                         xb3t[:, nb, csl], start=False,
                                         stop=False)
                    for nb in range(2):
                        nc.tensor.matmul(pm[:, nb, :],
                                         wm[:, 0, 256 + 128 * nb:
                                            256 + 128 * (nb + 1)],
                                         xreg[:, 0, csl], start=False,
                                         stop=False)
                        nc.tensor.matmul(pm[:, nb, :],
                                         wm[:, 1, 256 + 128 * nb:
                                            256 + 128 * (nb + 1)],
                                         xreg[:, 1, csl], start=False,
                                         stop=True)

                # ACT: CNN ladder for group g (must not sit behind evac)
                if g < NG:
                    eh1 = cnnpool.tile([128, G], f32, tag="eh1",
                                       name=f"eh1g{g}")
                    nc.scalar.activation(eh1[:], p1[:], AF.Exp, bias=b2v[:])
                    h1s = cnnpool.tile([128, G], f16, tag="h1s",
                                       name=f"h1sg{g}")
                    nc.scalar.activation(h1s[:], eh1[:], AF.Ln, bias=1.0)
                    p2 = pc23pool.tile([32, G], f32, tag="p23",
                                       name=f"p2g{g}")
                    nc.tensor.matmul(p2[:], m2bd[:], h1s[:],
                                     start=True, stop=True)
                    eh2 = cnnpool.tile([32, G], f32, tag="eh2",
                                       name=f"eh2g{g}")
                    nc.scalar.activation(eh2[:], p2[:], AF.Exp, bias=b3v[:])
                    h2s = cnnpool.tile([32, G], f16, tag="h2s",
                                       name=f"h2sg{g}")
                    nc.scalar.activation(h2s[:], eh2[:], AF.Ln, bias=1.0)
                    p3 = pc23pool.tile([2, G], f32, tag="p23",
                                       name=f"p3g{g}")
                    nc.tensor.matmul(p3[:], lwbd[:], h2s[:],
                                     start=True, stop=True)
                    spE = cnnpool.tile([2, G], f32, tag="spE",
                                       name=f"spEg{g}")
                    nc.scalar.activation(spE[:], p3[:], AF.Exp,
                                         bias=lbmr[:])
                    nc.scalar.activation(sp[:, gsl], spE[:], AF.Ln,
                                         bias=1.0)
                    # mu row is on partition 1 — broadcast sources must
                    # start at partition 0; hop via DMA
                    nc.sync.dma_start(murow[:, gsl], sp[1:2, gsl])

                # ACT: s evacuation PSUM -> SBUF f16 for chunk g-1
                if 1 <= g <= NG:
                    c = g - 1
                    s16 = s16pool.tile([128, 2 * G], f16, tag="s16",
                                       name=f"s16c{c}")
                    s16_t[c] = s16
                    nc.scalar.activation(
                        s16[:], pm_t[c][:].rearrange("p a b -> p (a b)"),
                        AF.Copy)

                # Vector: hm/q for chunk g-2 (from evacuated s)
                if 2 <= g <= NG + 1:
                    c = g - 2
                    csl = csl_of(c)
                    hm = hqpool.tile([128, 2, G], f16, tag="hm",
                                     name=f"hm{c}")
                    q = hqpool.tile([128, 2, G], f16, tag="q", name=f"q{c}")
                    hm_t[c] = hm
                    q_t[c] = q
                    for nb in range(2):
                        nsl = slice(G * nb, G * (nb + 1))
                        nc.vector._custom_dve(
                            COPS['HM'], out=hm[:, nb, :],
                            in0=s16_t[c][:, nsl], in1=mub[:, csl],
                            s0=gm2 * 2.0 / 3.0, s1=1.0 / 12.0, imm2=0.5)
                        nc.vector._custom_dve(
                            COPS['QQ'], out=q[:, nb, :],
                            in0=s16_t[c][:, nsl], in1=mub[:, csl],
                            s0=gm2, s1=-0.25, imm2=0.5)

                # ACT: ihm3 = 1/(3 hm) via exp(-ln(3 hm)) for chunk g-2
                if 2 <= g <= NG + 1:
                    c = g - 2
                    lh = midpool.tile([128, 2 * G], f16, tag="lh",
                                      name=f"lh{c}")
                    nc.scalar.activation(
                        lh[:], hm_t[c][:].rearrange("p a b -> p (a b)"),
                        AF.Ln, scale=3.0)
                    ihm3 = midpool.tile([128, 2 * G], f16, tag="ihm3",
                                        name=f"ihm3{c}")
                    nc.scalar.activation(ihm3[:], lh[:], AF.Exp, scale=-1.0)
                    ihm3_t[c] = ihm3

                # Vector: sigma chain for chunk g-3
                if 3 <= g <= NG + 2:
                    c = g - 3
                    qf = q_t[c][:].rearrange("p a b -> p (a b)")
                    e = midpool.tile([128, 2 * G], f16, tag="e",
                                     name=f"e{c}")
                    nc.vector._custom_dve(
                        COPS['SIGE'], out=e[:], in0=qf, in1=ihm3_t[c][:],
                        s0=SIG_C0, s1=SIG_C1, imm2=SIG_C2)
                    H = midpool.tile([128, 2 * G], f16, tag="H",
                                     name=f"H{c}")
                    H_t[c] = H
                    nc.vector._custom_dve(COPS['QE3'], out=H[:], in0=qf,
                                          in1=e[:], s0=1.0)
                    rg = midpool.tile([128, 2 * G], f16, tag="rg",
                                      name=f"rg{c}")
                    rg_t[c] = rg
                    nc.vector.tensor_tensor(rg[:], H[:], ihm3_t[c][:],
                                            Alu.mult)


    nc.compile()
    return nc


def _get_program(gamma, m):
    key = (B_TOTAL, N, N_CORES, float(gamma), float(m))
    if key not in _PROG:
        _PROG[key] = _build_program(gamma, m)
    return _PROG[key]


def _host_prep(inputs):
    f16 = np.float16
    x = _np_f32(inputs['x']).reshape(B_TOTAL, N)
    x_b = _np_f32(inputs['x_b']).reshape(B_TOTAL, N)
    m = float(np.asarray(inputs['mass']).reshape(-1)[0])
    gp = float(np.asarray(inputs['gamma_p']).reshape(-1)[0])
    gamma = float(np.log1p(np.exp(gp))) if gp < 30 else gp
    TtT = _np_f32(inputs['TtT'])
    DtD = _np_f32(inputs['DtD'])

    W_A = ((np.eye(N, dtype=np.float32) - np.float32(gamma) * TtT.T)
           / np.float32(3.0 * m))
    W_B = -np.float32(gamma) * DtD.T / np.float32(3.0 * m)
    WM = np.ascontiguousarray(
        np.concatenate([W_A, W_B], axis=1).astype(f16))        # (256,512)

    M1s, M2s, lws = {}, {}, {}
    for tag in ('mu', 'reg'):
        M1s[tag] = _conv_pool_mat(inputs['w2_' + tag], 256)
        M2s[tag] = _conv_pool_mat(inputs['w3_' + tag], 64)
        lws[tag] = _np_f32(inputs['lw_' + tag]).reshape(16)
    M1cat = np.concatenate([M1s['mu'], M1s['reg']], axis=0)     # (128,256)
    M1T = np.ascontiguousarray(M1cat.T.astype(f16))             # (256,128)
    M2BD = np.zeros((128, 32), f16)
    M2BD[0:64, 0:16] = M2s['mu'].T.astype(f16)
    M2BD[64:128, 16:32] = M2s['reg'].T.astype(f16)
    # column 0 -> reg (broadcast straight from partition 0), column 1 -> mu
    LWBD = np.zeros((32, 2), f16)
    LWBD[16:32, 0] = lws['reg'].astype(f16)
    LWBD[0:16, 1] = lws['mu'].astype(f16)

    def sc(name):
        return float(np.asarray(inputs[name]).reshape(-1)[0])

    B2V = np.full((128, 1), sc('b2_mu'), np.float32)
    B2V[64:] = sc('b2_reg')
    B3V = np.full((32, 1), sc('b3_mu'), np.float32)
    B3V[16:] = sc('b3_reg')
    LBMR = np.array([[sc('lb_reg')], [sc('lb_mu')]], np.float32)
    I128 = np.eye(128, dtype=f16)

    consts = dict(wm=WM, i128=I128, m1t=M1T, m2bd=M2BD, lwbd=LWBD,
                  b2v=B2V, b3v=B3V, lbmr=LBMR)

    xb3 = (np.float32(gamma / (3.0 * m)) * x_b
           + np.float32(1.0 / 3.0)).astype(np.float32)
    in_maps = []
    for c in range(N_CORES):
        rows = slice(BC * c, BC * (c + 1))
        im = dict(consts)
        im['xt'] = np.ascontiguousarray(x[rows].T.astype(f16))
        im['xb3t'] = np.ascontiguousarray(xb3[rows].T.astype(f16))
        in_maps.append(im)
    return in_maps, m, gamma


def kernel(**inputs) -> np.ndarray:
    from concourse import bass_utils
    in_maps, m, gamma = _host_prep(inputs)
    nc = _get_program(gamma, m)
    res = bass_utils.run_bass_kernel_spmd(nc, in_maps,
                                          core_ids=list(range(N_CORES)))
    parts = [res.results[c]['out'].T.astype(np.float32)
             for c in range(N_CORES)]
    out = np.concatenate(parts, axis=0)
    if m != 1.0:
        out = (np.float32(m) * out).astype(np.float32)
    return np.ascontiguousarray(out.reshape(B_TOTAL, 1, N))


# revision 26
# speedup vs baseline: 1.3037x; 1.3037x over previous
"""Trainium2 Bass kernel for nn_Block_34067680592489.

Computes, for B=32768 independent signals x[b] (length 256):
  mu,reg = small-CNN(x[b])      (conv5+avgpool4+softplus twice, linear, softplus)
  grad   = TtT x - x_b + reg * DtD x
  x_t    = x - gamma * grad,  gamma = softplus(gamma_p)
  out    = middle root of z^3 -(m+x_t) z^2 + (m x_t - 2 gm) z + gm m,  gm = gamma*mu

Device algorithm (signal-on-partitions layout, all matmuls fp16):
  s[n,b]  = x W_A + (reg*x) W_B + xb3            (single PSUM accumulation:
            W_A = (I-g TtT^T)/3m, W_B = -g DtD^T/3m, xb3 = g/(3m) x_b + 1/3;
            reg-fold via pre-scaled moving operand, xb3-fold via identity-
            block stationary operand)
  hm   = (s-1/2)^2 + (2/3) gh + 1/4              (DVE fused)
  q    = ((s-1/2)^2 + gh) * (1-2s)               (DVE fused)
  ihm3 = 1/(3 hm)                                 (ACT reciprocal table)
  e    = y'*P(t'), y'=q*ihm3, t'=y'^2*ihm3        (DVE fused; P = deg-2 poly
          fitted so e ~= r*sin(arcsin(w)/3), w = q/(2 hm^1.5), r = 2 sqrt(hm))
  H    = q + e^3                                  (DVE fused; one fixed-point
          refinement of the triple-angle identity 3 sigma - 4 sigma^3 = w)
  out  = s + H * ihm3                             (GpSimd mult + DVE add)
No ln/exp/trig at all -> single ACT table switch (softplus CNN via exp/ln
happens entirely before the reciprocal phase).

Sharding: pure data parallel over batch, 8 cores x 4096 rows.
"""

import numpy as np

B_TOTAL = 32768
N = 256
N_CORES = 8
BC = B_TOTAL // N_CORES      # rows per core (4096)
G = 512                      # rows per group/chunk
NG = BC // G                 # 8 groups

# deg-2 odd-poly fit of sin(arcsin(w)/3)/w on w in [0, 0.825], rescaled for
# the t' = y'^2*ihm3 argument (y' = q*ihm3, ihm3 = 1/(3 hm)).
SIG_C0 = 1.0007408750509472
SIG_C1 = 0.8616695104467121
SIG_C2 = 6.336400405433818

_PROG = {}


def _np_f32(a):
    return np.ascontiguousarray(np.asarray(a, dtype=np.float32))


def _conv_pool_mat(w, L):
    """(L/4, L) matrix implementing conv1d(k=5,pad=2) then avgpool4."""
    taps = np.asarray(w, np.float32).reshape(5)
    C = np.zeros((L, L), np.float32)
    for n in range(L):
        for k in range(5):
            m = n + k - 2
            if 0 <= m < L:
                C[n, m] = taps[k]
    P = np.zeros((L // 4, L), np.float32)
    for i in range(L // 4):
        P[i, 4 * i:4 * i + 4] = 0.25
    return (P @ C).astype(np.float32)


_CUSTOM_OPS = {}


def _get_custom_ops():
    """Register this kernel's fused custom-DVE ops (idempotent)."""
    if _CUSTOM_OPS:
        return _CUSTOM_OPS
    import concourse.dve_ops as dops
    from concourse.dve_spec import (Spec, Src0, Src1, C0, C1, C2, One,
                                    sq, lower, _has_src1)
    from concourse.dve_uop import DveOpSpec

    def reg(name, spec):
        if name in dops._SUB_OPCODE_FOR_NAME:
            return next(o for o in dops.OPS if o.name == name)
        row = dops._CUSTOM_DVE_ROW_BASE + len(dops.OPS)
        assert row < 0x20
        dops._SUB_OPCODE_FOR_NAME[name] = row
        shas = {}
        for ver in ("v3", "v4"):
            u = lower(spec, ver=ver)
            shas[ver] = DveOpSpec(name=name, opcode=row, uops=u,
                                  rd1_en=_has_src1(spec)).sha(ver)
        op = dops.DveOp(name, spec, subdim=False, uops_sha=shas)
        dops.OPS.append(op)
        dops.CUSTOM_DVE_SPECS[name] = spec
        return op

    import numpy as np_

    # hm = (s-imm2)^2 + (gh*s0 + s1)
    _CUSTOM_OPS['HM'] = reg('ANT_K_HM2', Spec(
        body=sq(Src0 - C2) + (Src1 * C0 + C1),
        reference=lambda in0, in1, s0, s1, imm2:
            ((in0 - imm2) ** 2 + (in1 * s0 + s1)).astype(np_.float32),
    ))
    # q = ((s-imm2)^2 + (mu*s0 + s1)) * (1 - 2 s)
    _CUSTOM_OPS['QQ'] = reg('ANT_K_QQ3', Spec(
        body=(sq(Src0 - C2) + (Src1 * C0 + C1)) * (One - (Src0 + Src0)),
        reference=lambda in0, in1, s0, s1, imm2:
            (((in0 - imm2) ** 2 + (in1 * s0 + s1)) * (1.0 - (in0 + in0))
             ).astype(np_.float32),
    ))
    # e = y*(C0 + t*(C1 + t*C2)), y = q*ihm3, t = y^2*ihm3
    _y = Src0 * Src1
    _t = sq(_y) * Src1
    _CUSTOM_OPS['SIGE'] = reg('ANT_K_SIGE', Spec(
        body=_y * (C0 + _t * (C1 + _t * C2)),
        reference=lambda in0, in1, s0, s1, imm2:
            ((in0 * in1) * (s0 + (in0 * in1) ** 2 * in1
                            * (s1 + (in0 * in1) ** 2 * in1 * imm2))
             ).astype(np_.float32),
    ))
    # H = (q + e^3) * s0
    _CUSTOM_OPS['QE3'] = reg('ANT_K_QE3B', Spec(
        body=(Src0 + sq(Src1) * Src1) * C0,
        reference=lambda in0, in1, s0, s1, imm2:
            ((in0 + in1 ** 3) * s0).astype(np_.float32),
    ))
    return _CUSTOM_OPS


_TABLES_PATCHED = False


def _patch_act_tables():
    """Restrict ACT table-set choice to the two sets this kernel uses:
    natural_log_exp (CNN softplus) and reciprocal (cardan 1/(3 hm))."""
    global _TABLES_PATCHED
    if _TABLES_PATCHED:
        return
    import concourse.bacc as bacc
    keep = {'natural_log_exp_and_others'}
    orig = bacc.get_activation_tables

    def patched(arch):
        t = orig(arch)
        return {k: (v if k in keep else set()) for k, v in t.items()}

    bacc.get_activation_tables = patched
    _TABLES_PATCHED = True


def _build_program(gamma, m):
    import concourse.bacc as bacc
    import concourse.tile as tile
    import concourse.mybir as mybir
    _patch_act_tables()

    dt = mybir.dt
    f32 = dt.float32
    f16 = dt.float16
    Alu = mybir.AluOpType
    AF = mybir.ActivationFunctionType

    COPS = _get_custom_ops()
    nc = bacc.Bacc("TRN2", target_bir_lowering=False, debug=False,
                   num_devices=N_CORES)

    XT = nc.dram_tensor("xt", (256, BC), f16, kind="ExternalInput")
    XB3T = nc.dram_tensor("xb3t", (256, BC), f16, kind="ExternalInput")
    WM = nc.dram_tensor("wm", (256, 512), f16, kind="ExternalInput")
    I128 = nc.dram_tensor("i128", (128, 128), f16, kind="ExternalInput")
    M1T = nc.dram_tensor("m1t", (256, 128), f16, kind="ExternalInput")
    M2BD = nc.dram_tensor("m2bd", (128, 32), f16, kind="ExternalInput")
    LWBD = nc.dram_tensor("lwbd", (32, 2), f16, kind="ExternalInput")
    B2V = nc.dram_tensor("b2v", (128, 1), f32, kind="ExternalInput")
    B3V = nc.dram_tensor("b3v", (32, 1), f32, kind="ExternalInput")
    LBMR = nc.dram_tensor("lbmr", (2, 1), f32, kind="ExternalInput")
    OUT = nc.dram_tensor("out", (256, BC), f16, kind="ExternalOutput")

    gm2 = float(gamma / (m * m))

    with tile.TileContext(nc) as tc:
        with (
            tc.tile_pool(name="const", bufs=1) as cpool,
            tc.tile_pool(name="xin", bufs=1) as xpool,
            tc.tile_pool(name="scal", bufs=1) as spool,
            tc.tile_pool(name="cnn", bufs=2) as cnnpool,
            tc.tile_pool(name="hq", bufs=4) as hqpool,
            tc.tile_pool(name="s16", bufs=6) as s16pool,
            tc.tile_pool(name="mid", bufs=3) as midpool,
            tc.tile_pool(name="oout", bufs=5) as opool,
            tc.tile_pool(name="pm", bufs=2, space="PSUM") as pmpool,
            tc.tile_pool(name="rpb", bufs=2, space="PSUM") as rpbpool,
            tc.tile_pool(name="pc1", bufs=1, space="PSUM") as pc1pool,
            tc.tile_pool(name="pc23", bufs=1, space="PSUM") as pc23pool,
        ):
            # ---- tiles ----
            wm = cpool.tile([128, 2, 512], f16)
            i128 = cpool.tile([128, 128], f16)
            m1t = cpool.tile([128, 2, 128], f16)
            m2bd = cpool.tile([128, 32], f16)
            lwbd = cpool.tile([32, 2], f16)
            b2v = cpool.tile([128, 1], f32)
            b3v = cpool.tile([32, 1], f32)
            lbmr = cpool.tile([2, 1], f32)
            xt = xpool.tile([128, 2, BC], f16)
            xb3t = xpool.tile([128, 2, BC], f16)
            xreg = xpool.tile([128, 2, BC], f16)
            sp = spool.tile([2, BC], f16)      # row0 = reg, row1 = mu
            murow = spool.tile([1, BC], f16)
            mub = spool.tile([128, BC], f16)
            ones1 = spool.tile([1, 128], f16)
            nc.vector.memset(ones1[:], 1.0)

            # ---- input DMAs, most-needed first ----
            for k in range(2):
                nc.sync.dma_start(m1t[:, k, :], M1T[128 * k:128 * (k + 1), :])
            for k in range(2):
                nc.sync.dma_start(xt[:, k, 0:BC // 4],
                                  XT[128 * k:128 * (k + 1), 0:BC // 4])
            for k in range(2):
                nc.sync.dma_start(wm[:, k, :], WM[128 * k:128 * (k + 1), :])
            nc.sync.dma_start(i128[:], I128[:])
            nc.sync.dma_start(m2bd[:], M2BD[:])
            nc.sync.dma_start(lwbd[:], LWBD[:])
            nc.sync.dma_start(b2v[:], B2V[:])
            nc.sync.dma_start(b3v[:], B3V[:])
            nc.sync.dma_start(lbmr[:], LBMR[:])
            for qq in range(4):
                qsl = slice(BC // 4 * qq, BC // 4 * (qq + 1))
                for k in range(2):
                    nc.sync.dma_start(xb3t[:, k, qsl],
                                      XB3T[128 * k:128 * (k + 1), qsl])
                if qq > 0:
                    for k in range(2):
                        nc.sync.dma_start(xt[:, k, qsl],
                                          XT[128 * k:128 * (k + 1), qsl])

            hm_t = [None] * NG
            q_t = [None] * NG
            s16_t = [None] * NG
            ihm3_t = [None] * NG
            H_t = [None] * NG
            rg_t = [None] * NG
            ot_t = [None] * NG
            pm_t = [None] * NG

            def csl_of(c):
                return slice(G * c, G * (c + 1))

            # Software-pipelined schedule; chunk c: main matmuls at iter
            # c+1, s evac c+1, hm/q + ln/exp c+2, sigma chain c+3, final +
            # store c+4. Queue order per iteration keeps the critical cycle
            # (CNN ladder -> sp -> broadcast -> xreg -> main matmuls) free
            # of unrelated head-of-line waits on every engine.
            for g in range(NG + 5):
                gsl = slice(G * g, G * (g + 1))

                # PE: CNN level-1 matmul for group g (always ready)
                if g < NG:
                    p1 = pc1pool.tile([128, G], f32, tag="p1",
                                      name=f"p1g{g}")
                    nc.tensor.matmul(p1[:], m1t[:, 0, :], xt[:, 0, gsl],
                                     start=True, stop=False)
                    nc.tensor.matmul(p1[:], m1t[:, 1, :], xt[:, 1, gsl],
                                     start=False, stop=True)

                # PE: broadcast reg row via rank-1 matmul (ones x reg)
                rpb = None
                if 1 <= g <= NG:
                    c = g - 1
                    rpb = rpbpool.tile([128, G], f32, tag="rpb",
                                       name=f"rpb{c}")
                    nc.tensor.matmul(rpb[:], ones1[:], sp[0:1, csl_of(c)],
                                     start=True, stop=True)
                # GpSimd: mu broadcast for g-1 (off critical path)
                if 1 <= g <= NG:
                    c = g - 1
                    nc.gpsimd.partition_broadcast(mub[:, csl_of(c)],
                                                  murow[:, csl_of(c)])

                # GpSimd: rg for chunk g-3 (after final below issues first)
                if 4 <= g < NG + 4:
                    c = g - 4
                    ot = opool.tile([128, 2, G], f16, tag="ot",
                                    name=f"ot{c}")
                    ot_t[c] = ot
                    nc.gpsimd.tensor_tensor(
                        ot[:].rearrange("p a b -> p (a b)"), s16_t[c][:],
                        rg_t[c][:], Alu.add)
                    dview = OUT.rearrange("(h p) r -> p h r",
                                          p=128)[:, :, csl_of(c)]
                    nc.sync.dma_start(dview, ot[:])

                # Vector: xreg for g-1 (critical), then trailing stages
                if 1 <= g <= NG:
                    c = g - 1
                    for k in range(2):
                        nc.vector.tensor_tensor(xreg[:, k, csl_of(c)],
                                                xt[:, k, csl_of(c)],
                                                rpb[:], Alu.mult)

                # PE: main matmuls for chunk g-1; the xreg-independent
                # A/identity parts are issued before the B parts so they
                # overlap the broadcast -> xreg chain.
                if 1 <= g <= NG:
                    c = g - 1
                    csl = csl_of(c)
                    pm = pmpool.tile([128, 2, G], f32, tag="pm",
                                     name=f"pm{c}")
                    pm_t[c] = pm
                    for nb in range(2):
                        nc.tensor.matmul(pm[:, nb, :],
                                         wm[:, 0, 128 * nb:128 * (nb + 1)],
                                         xt[:, 0, csl], start=True,
                                         stop=False)
                        nc.tensor.matmul(pm[:, nb, :],
                                         wm[:, 1, 128 * nb:128 * (nb + 1)],
                                         xt[:, 1, csl], start=False,
                                         stop=False)
                        nc.tensor.matmul(pm[:, nb, :],
                                         wm[:, 0, 256 + 128 * nb:
                                            256 + 128 * (nb + 1)],
                                         xreg[:, 0, csl], start=False,
                                         stop=False)
                        nc.tensor.matmul(pm[:, nb, :],
                                         wm[:, 1, 256 + 128 * nb:
                                            256 + 128 * (nb + 1)],
                                         xreg[:, 1, csl], start=False,
                                         stop=False)
                        nc.tensor.matmul(pm[:, nb, :], i128[:],
                                         xb3t[:, nb, csl], start=False,
                                         stop=True)

                # ACT: CNN ladder for group g (must not sit behind evac)
                if g < NG:
                    eh1 = cnnpool.tile([128, G], f32, tag="eh1",
                                       name=f"eh1g{g}")
                    nc.scalar.activation(eh1[:], p1[:], AF.Exp, bias=b2v[:])
                    h1s = cnnpool.tile([128, G], f16, tag="h1s",
                                       name=f"h1sg{g}")
                    nc.scalar.activation(h1s[:], eh1[:], AF.Ln, bias=1.0)
                    p2 = pc23pool.tile([32, G], f32, tag="p23",
                                       name=f"p2g{g}")
                    nc.tensor.matmul(p2[:], m2bd[:], h1s[:],
                                     start=True, stop=True)
                    eh2 = cnnpool.tile([32, G], f32, tag="eh2",
                                       name=f"eh2g{g}")
                    nc.scalar.activation(eh2[:], p2[:], AF.Exp, bias=b3v[:])
                    h2s = cnnpool.tile([32, G], f16, tag="h2s",
                                       name=f"h2sg{g}")
                    nc.scalar.activation(h2s[:], eh2[:], AF.Ln, bias=1.0)
                    p3 = pc23pool.tile([2, G], f32, tag="p23",
                                       name=f"p3g{g}")
                    nc.tensor.matmul(p3[:], lwbd[:], h2s[:],
                                     start=True, stop=True)
                    spE = cnnpool.tile([2, G], f32, tag="spE",
                                       name=f"spEg{g}")
                    nc.scalar.activation(spE[:], p3[:], AF.Exp,
                                         bias=lbmr[:])
                    nc.scalar.activation(sp[:, gsl], spE[:], AF.Ln,
                                         bias=1.0)
                    # mu row is on partition 1 — broadcast sources must
                    # start at partition 0; hop via DMA
                    nc.sync.dma_start(murow[:, gsl], sp[1:2, gsl])

                # ACT: s evacuation PSUM -> SBUF f16 for chunk g-1
                if 1 <= g <= NG:
                    c = g - 1
                    s16 = s16pool.tile([128, 2 * G], f16, tag="s16",
                                       name=f"s16c{c}")
                    s16_t[c] = s16
                    nc.scalar.activation(
                        s16[:], pm_t[c][:].rearrange("p a b -> p (a b)"),
                        AF.Copy)

                # Vector: hm/q for chunk g-2 (from evacuated s)
                if 2 <= g <= NG + 1:
                    c = g - 2
                    csl = csl_of(c)
                    hm = hqpool.tile([128, 2, G], f16, tag="hm",
                                     name=f"hm{c}")
                    q = hqpool.tile([128, 2, G], f16, tag="q", name=f"q{c}")
                    hm_t[c] = hm
                    q_t[c] = q
                    for nb in range(2):
                        nsl = slice(G * nb, G * (nb + 1))
                        nc.vector._custom_dve(
                            COPS['HM'], out=hm[:, nb, :],
                            in0=s16_t[c][:, nsl], in1=mub[:, csl],
                            s0=gm2 * 2.0 / 3.0, s1=1.0 / 12.0, imm2=0.5)
                        nc.vector._custom_dve(
                            COPS['QQ'], out=q[:, nb, :],
                            in0=s16_t[c][:, nsl], in1=mub[:, csl],
                            s0=gm2, s1=-0.25, imm2=0.5)

                # ACT: ihm3 = 1/(3 hm) via exp(-ln(3 hm)) for chunk g-2
                if 2 <= g <= NG + 1:
                    c = g - 2
                    lh = midpool.tile([128, 2 * G], f16, tag="lh",
                                      name=f"lh{c}")
                    nc.scalar.activation(
                        lh[:], hm_t[c][:].rearrange("p a b -> p (a b)"),
                        AF.Ln, scale=3.0)
                    ihm3 = midpool.tile([128, 2 * G], f16, tag="ihm3",
                                        name=f"ihm3{c}")
                    nc.scalar.activation(ihm3[:], lh[:], AF.Exp, scale=-1.0)
                    ihm3_t[c] = ihm3

                # Vector: sigma chain for chunk g-3
                if 3 <= g <= NG + 2:
                    c = g - 3
                    qf = q_t[c][:].rearrange("p a b -> p (a b)")
                    e = midpool.tile([128, 2 * G], f16, tag="e",
                                     name=f"e{c}")
                    nc.vector._custom_dve(
                        COPS['SIGE'], out=e[:], in0=qf, in1=ihm3_t[c][:],
                        s0=SIG_C0, s1=SIG_C1, imm2=SIG_C2)
                    H = midpool.tile([128, 2 * G], f16, tag="H",
                                     name=f"H{c}")
                    H_t[c] = H
                    nc.vector._custom_dve(COPS['QE3'], out=H[:], in0=qf,
                                          in1=e[:], s0=1.0)
                    rg = midpool.tile([128, 2 * G], f16, tag="rg",
                                      name=f"rg{c}")
                    rg_t[c] = rg
                    nc.vector.tensor_tensor(rg[:], H[:], ihm3_t[c][:],
                                            Alu.mult)


    nc.compile()
    return nc


def _get_program(gamma, m):
    key = (B_TOTAL, N, N_CORES, float(gamma), float(m))
    if key not in _PROG:
        _PROG[key] = _build_program(gamma, m)
    return _PROG[key]


def _host_prep(inputs):
    f16 = np.float16
    x = _np_f32(inputs['x']).reshape(B_TOTAL, N)
    x_b = _np_f32(inputs['x_b']).reshape(B_TOTAL, N)
    m = float(np.asarray(inputs['mass']).reshape(-1)[0])
    gp = float(np.asarray(inputs['gamma_p']).reshape(-1)[0])
    gamma = float(np.log1p(np.exp(gp))) if gp < 30 else gp
    TtT = _np_f32(inputs['TtT'])
    DtD = _np_f32(inputs['DtD'])

    W_A = ((np.eye(N, dtype=np.float32) - np.float32(gamma) * TtT.T)
           / np.float32(3.0 * m))
    W_B = -np.float32(gamma) * DtD.T / np.float32(3.0 * m)
    WM = np.ascontiguousarray(
        np.concatenate([W_A, W_B], axis=1).astype(f16))        # (256,512)

    M1s, M2s, lws = {}, {}, {}
    for tag in ('mu', 'reg'):
        M1s[tag] = _conv_pool_mat(inputs['w2_' + tag], 256)
        M2s[tag] = _conv_pool_mat(inputs['w3_' + tag], 64)
        lws[tag] = _np_f32(inputs['lw_' + tag]).reshape(16)
    M1cat = np.concatenate([M1s['mu'], M1s['reg']], axis=0)     # (128,256)
    M1T = np.ascontiguousarray(M1cat.T.astype(f16))             # (256,128)
    M2BD = np.zeros((128, 32), f16)
    M2BD[0:64, 0:16] = M2s['mu'].T.astype(f16)
    M2BD[64:128, 16:32] = M2s['reg'].T.astype(f16)
    # column 0 -> reg (broadcast straight from partition 0), column 1 -> mu
    LWBD = np.zeros((32, 2), f16)
    LWBD[16:32, 0] = lws['reg'].astype(f16)
    LWBD[0:16, 1] = lws['mu'].astype(f16)

    def sc(name):
        return float(np.asarray(inputs[name]).reshape(-1)[0])

    B2V = np.full((128, 1), sc('b2_mu'), np.float32)
    B2V[64:] = sc('b2_reg')
    B3V = np.full((32, 1), sc('b3_mu'), np.float32)
    B3V[16:] = sc('b3_reg')
    LBMR = np.array([[sc('lb_reg')], [sc('lb_mu')]], np.float32)
    I128 = np.eye(128, dtype=f16)

    consts = dict(wm=WM, i128=I128, m1t=M1T, m2bd=M2BD, lwbd=LWBD,
                  b2v=B2V, b3v=B3V, lbmr=LBMR)

    xb3 = (np.float32(gamma / (3.0 * m)) * x_b
           + np.float32(1.0 / 3.0)).astype(np.float32)
    in_maps = []
    for c in range(N_CORES):
        rows = slice(BC * c, BC * (c + 1))
        im = dict(consts)
        im['xt'] = np.ascontiguousarray(x[rows].T.astype(f16))
        im['xb3t'] = np.ascontiguousarray(xb3[rows].T.astype(f16))
        in_maps.append(im)
    return in_maps, m, gamma


def kernel(**inputs) -> np.ndarray:
    from concourse import bass_utils
    in_maps, m, gamma = _host_prep(inputs)
    nc = _get_program(gamma, m)
    res = bass_utils.run_bass_kernel_spmd(nc, in_maps,
                                          core_ids=list(range(N_CORES)))
    parts = [res.results[c]['out'].T.astype(np.float32)
             for c in range(N_CORES)]
    out = np.concatenate(parts, axis=0)
    if m != 1.0:
        out = (np.float32(m) * out).astype(np.float32)
    return np.ascontiguousarray(out.reshape(B_TOTAL, 1, N))


# revision 27
# speedup vs baseline: 1.3290x; 1.0194x over previous
"""Trainium2 Bass kernel for nn_Block_34067680592489.

Computes, for B=32768 independent signals x[b] (length 256):
  mu,reg = small-CNN(x[b])      (conv5+avgpool4+softplus twice, linear, softplus)
  grad   = TtT x - x_b + reg * DtD x
  x_t    = x - gamma * grad,  gamma = softplus(gamma_p)
  out    = middle root of z^3 -(m+x_t) z^2 + (m x_t - 2 gm) z + gm m,  gm = gamma*mu

Device algorithm (signal-on-partitions layout, all matmuls fp16):
  s[n,b]  = x W_A + (reg*x) W_B + xb3            (single PSUM accumulation:
            W_A = (I-g TtT^T)/3m, W_B = -g DtD^T/3m, xb3 = g/(3m) x_b + 1/3;
            reg-fold via pre-scaled moving operand, xb3-fold via identity-
            block stationary operand)
  hm   = (s-1/2)^2 + (2/3) gh + 1/4              (DVE fused)
  q    = ((s-1/2)^2 + gh) * (1-2s)               (DVE fused)
  ihm3 = 1/(3 hm)                                 (ACT reciprocal table)
  e    = y'*P(t'), y'=q*ihm3, t'=y'^2*ihm3        (DVE fused; P = deg-2 poly
          fitted so e ~= r*sin(arcsin(w)/3), w = q/(2 hm^1.5), r = 2 sqrt(hm))
  H    = q + e^3                                  (DVE fused; one fixed-point
          refinement of the triple-angle identity 3 sigma - 4 sigma^3 = w)
  out  = s + H * ihm3                             (GpSimd mult + DVE add)
No ln/exp/trig at all -> single ACT table switch (softplus CNN via exp/ln
happens entirely before the reciprocal phase).

Sharding: pure data parallel over batch, 8 cores x 4096 rows.
"""

import numpy as np

B_TOTAL = 32768
N = 256
N_CORES = 8
BC = B_TOTAL // N_CORES      # rows per core (4096)
G = 512                      # rows per group/chunk
NG = BC // G                 # 8 groups

# deg-2 odd-poly fit of sin(arcsin(w)/3)/w on w in [0, 0.825], rescaled for
# the t' = y'^2*ihm3 argument (y' = q*ihm3, ihm3 = 1/(3 hm)).
SIG_C0 = 1.0007408750509472
SIG_C1 = 0.8616695104467121
SIG_C2 = 6.336400405433818

_PROG = {}


def _np_f32(a):
    return np.ascontiguousarray(np.asarray(a, dtype=np.float32))


def _conv_pool_mat(w, L):
    """(L/4, L) matrix implementing conv1d(k=5,pad=2) then avgpool4."""
    taps = np.asarray(w, np.float32).reshape(5)
    C = np.zeros((L, L), np.float32)
    for n in range(L):
        for k in range(5):
            m = n + k - 2
            if 0 <= m < L:
                C[n, m] = taps[k]
    P = np.zeros((L // 4, L), np.float32)
    for i in range(L // 4):
        P[i, 4 * i:4 * i + 4] = 0.25
    return (P @ C).astype(np.float32)


_CUSTOM_OPS = {}


def _get_custom_ops():
    """Register this kernel's fused custom-DVE ops (idempotent)."""
    if _CUSTOM_OPS:
        return _CUSTOM_OPS
    import concourse.dve_ops as dops
    from concourse.dve_spec import (Spec, Src0, Src1, C0, C1, C2, One,
                                    sq, lower, _has_src1)
    from concourse.dve_uop import DveOpSpec

    def reg(name, spec):
        if name in dops._SUB_OPCODE_FOR_NAME:
            return next(o for o in dops.OPS if o.name == name)
        row = dops._CUSTOM_DVE_ROW_BASE + len(dops.OPS)
        assert row < 0x20
        dops._SUB_OPCODE_FOR_NAME[name] = row
        shas = {}
        for ver in ("v3", "v4"):
            u = lower(spec, ver=ver)
            shas[ver] = DveOpSpec(name=name, opcode=row, uops=u,
                                  rd1_en=_has_src1(spec)).sha(ver)
        op = dops.DveOp(name, spec, subdim=False, uops_sha=shas)
        dops.OPS.append(op)
        dops.CUSTOM_DVE_SPECS[name] = spec
        return op

    import numpy as np_

    # hm = (s-imm2)^2 + (gh*s0 + s1)
    _CUSTOM_OPS['HM'] = reg('ANT_K_HM2', Spec(
        body=sq(Src0 - C2) + (Src1 * C0 + C1),
        reference=lambda in0, in1, s0, s1, imm2:
            ((in0 - imm2) ** 2 + (in1 * s0 + s1)).astype(np_.float32),
    ))
    # q = ((s-imm2)^2 + (mu*s0 + s1)) * (1 - 2 s)
    _CUSTOM_OPS['QQ'] = reg('ANT_K_QQ3', Spec(
        body=(sq(Src0 - C2) + (Src1 * C0 + C1)) * (One - (Src0 + Src0)),
        reference=lambda in0, in1, s0, s1, imm2:
            (((in0 - imm2) ** 2 + (in1 * s0 + s1)) * (1.0 - (in0 + in0))
             ).astype(np_.float32),
    ))
    # e = y*(C0 + t*(C1 + t*C2)), y = q*ihm3, t = y^2*ihm3
    _y = Src0 * Src1
    _t = sq(_y) * Src1
    _CUSTOM_OPS['SIGE'] = reg('ANT_K_SIGE', Spec(
        body=_y * (C0 + _t * (C1 + _t * C2)),
        reference=lambda in0, in1, s0, s1, imm2:
            ((in0 * in1) * (s0 + (in0 * in1) ** 2 * in1
                            * (s1 + (in0 * in1) ** 2 * in1 * imm2))
             ).astype(np_.float32),
    ))
    # H = (q + e^3) * s0
    _CUSTOM_OPS['QE3'] = reg('ANT_K_QE3B', Spec(
        body=(Src0 + sq(Src1) * Src1) * C0,
        reference=lambda in0, in1, s0, s1, imm2:
            ((in0 + in1 ** 3) * s0).astype(np_.float32),
    ))
    return _CUSTOM_OPS


_TABLES_PATCHED = False


def _patch_act_tables():
    """Restrict ACT table-set choice to the two sets this kernel uses:
    natural_log_exp (CNN softplus) and reciprocal (cardan 1/(3 hm))."""
    global _TABLES_PATCHED
    if _TABLES_PATCHED:
        return
    import concourse.bacc as bacc
    keep = {'natural_log_exp_and_others'}
    orig = bacc.get_activation_tables

    def patched(arch):
        t = orig(arch)
        return {k: (v if k in keep else set()) for k, v in t.items()}

    bacc.get_activation_tables = patched
    _TABLES_PATCHED = True


def _build_program(gamma, m):
    import concourse.bacc as bacc
    import concourse.tile as tile
    import concourse.mybir as mybir
    _patch_act_tables()

    dt = mybir.dt
    f32 = dt.float32
    f16 = dt.float16
    Alu = mybir.AluOpType
    AF = mybir.ActivationFunctionType

    COPS = _get_custom_ops()
    nc = bacc.Bacc("TRN2", target_bir_lowering=False, debug=False,
                   num_devices=N_CORES)

    XT = nc.dram_tensor("xt", (256, BC), f16, kind="ExternalInput")
    XB3T = nc.dram_tensor("xb3t", (256, BC), f16, kind="ExternalInput")
    WM = nc.dram_tensor("wm", (256, 512), f16, kind="ExternalInput")
    I128 = nc.dram_tensor("i128", (128, 128), f16, kind="ExternalInput")
    M1T = nc.dram_tensor("m1t", (256, 128), f16, kind="ExternalInput")
    M2BD = nc.dram_tensor("m2bd", (128, 32), f16, kind="ExternalInput")
    LWBD = nc.dram_tensor("lwbd", (32, 2), f16, kind="ExternalInput")
    B2V = nc.dram_tensor("b2v", (128, 1), f32, kind="ExternalInput")
    B3V = nc.dram_tensor("b3v", (32, 1), f32, kind="ExternalInput")
    LBMR = nc.dram_tensor("lbmr", (2, 1), f32, kind="ExternalInput")
    OUT = nc.dram_tensor("out", (256, BC), f16, kind="ExternalOutput")

    gm2 = float(gamma / (m * m))

    with tile.TileContext(nc) as tc:
        with (
            tc.tile_pool(name="const", bufs=1) as cpool,
            tc.tile_pool(name="xin", bufs=1) as xpool,
            tc.tile_pool(name="scal", bufs=1) as spool,
            tc.tile_pool(name="cnn", bufs=2) as cnnpool,
            tc.tile_pool(name="hq", bufs=5) as hqpool,
            tc.tile_pool(name="s16", bufs=8) as s16pool,
            tc.tile_pool(name="mid", bufs=4) as midpool,
            tc.tile_pool(name="oout", bufs=7) as opool,
            tc.tile_pool(name="pm", bufs=2, space="PSUM") as pmpool,
            tc.tile_pool(name="rpb", bufs=2, space="PSUM") as rpbpool,
            tc.tile_pool(name="pc1", bufs=1, space="PSUM") as pc1pool,
            tc.tile_pool(name="pc23", bufs=1, space="PSUM") as pc23pool,
        ):
            # ---- tiles ----
            wm = cpool.tile([128, 2, 512], f16)
            i128 = cpool.tile([128, 128], f16)
            m1t = cpool.tile([128, 2, 128], f16)
            m2bd = cpool.tile([128, 32], f16)
            lwbd = cpool.tile([32, 2], f16)
            b2v = cpool.tile([128, 1], f32)
            b3v = cpool.tile([32, 1], f32)
            lbmr = cpool.tile([2, 1], f32)
            xt = xpool.tile([128, 2, BC], f16)
            xb3t = xpool.tile([128, 2, BC], f16)
            xreg = xpool.tile([128, 2, BC], f16)
            sp = spool.tile([2, BC], f16)      # row0 = reg, row1 = mu
            murow = spool.tile([1, BC], f16)
            mub = spool.tile([128, BC], f16)
            ones1 = spool.tile([1, 128], f16)
            nc.vector.memset(ones1[:], 1.0)

            # ---- input DMAs, most-needed first ----
            for k in range(2):
                nc.sync.dma_start(m1t[:, k, :], M1T[128 * k:128 * (k + 1), :])
            for k in range(2):
                nc.sync.dma_start(xt[:, k, 0:BC // 4],
                                  XT[128 * k:128 * (k + 1), 0:BC // 4])
            for k in range(2):
                nc.sync.dma_start(wm[:, k, :], WM[128 * k:128 * (k + 1), :])
            nc.sync.dma_start(i128[:], I128[:])
            nc.sync.dma_start(m2bd[:], M2BD[:])
            nc.sync.dma_start(lwbd[:], LWBD[:])
            nc.sync.dma_start(b2v[:], B2V[:])
            nc.sync.dma_start(b3v[:], B3V[:])
            nc.sync.dma_start(lbmr[:], LBMR[:])
            for qq in range(4):
                qsl = slice(BC // 4 * qq, BC // 4 * (qq + 1))
                for k in range(2):
                    nc.sync.dma_start(xb3t[:, k, qsl],
                                      XB3T[128 * k:128 * (k + 1), qsl])
                if qq > 0:
                    for k in range(2):
                        nc.sync.dma_start(xt[:, k, qsl],
                                          XT[128 * k:128 * (k + 1), qsl])

            hm_t = [None] * NG
            q_t = [None] * NG
            s16_t = [None] * NG
            ihm3_t = [None] * NG
            H_t = [None] * NG
            rg_t = [None] * NG
            ot_t = [None] * NG
            pm_t = [None] * NG

            def csl_of(c):
                return slice(G * c, G * (c + 1))

            # Software-pipelined schedule; chunk c: main matmuls at iter
            # c+1, s evac c+1, hm/q + ln/exp c+2, sigma chain c+3, final +
            # store c+4. Queue order per iteration keeps the critical cycle
            # (CNN ladder -> sp -> broadcast -> xreg -> main matmuls) free
            # of unrelated head-of-line waits on every engine.
            for g in range(NG + 5):
                gsl = slice(G * g, G * (g + 1))

                # PE: CNN level-1 matmul for group g (always ready)
                if g < NG:
                    p1 = pc1pool.tile([128, G], f32, tag="p1",
                                      name=f"p1g{g}")
                    nc.tensor.matmul(p1[:], m1t[:, 0, :], xt[:, 0, gsl],
                                     start=True, stop=False)
                    nc.tensor.matmul(p1[:], m1t[:, 1, :], xt[:, 1, gsl],
                                     start=False, stop=True)

                # PE: broadcast reg row via rank-1 matmul (ones x reg)
                rpb = None
                if 1 <= g <= NG:
                    c = g - 1
                    rpb = rpbpool.tile([128, G], f32, tag="rpb",
                                       name=f"rpb{c}")
                    nc.tensor.matmul(rpb[:], ones1[:], sp[0:1, csl_of(c)],
                                     start=True, stop=True)
                # GpSimd: mu broadcast for g-1 (off critical path)
                if 1 <= g <= NG:
                    c = g - 1
                    nc.gpsimd.partition_broadcast(mub[:, csl_of(c)],
                                                  murow[:, csl_of(c)])

                # GpSimd: rg for chunk g-3 (after final below issues first)
                if 4 <= g < NG + 4:
                    c = g - 4
                    ot = opool.tile([128, 2, G], f16, tag="ot",
                                    name=f"ot{c}")
                    ot_t[c] = ot
                    nc.gpsimd.tensor_tensor(
                        ot[:].rearrange("p a b -> p (a b)"), s16_t[c][:],
                        rg_t[c][:], Alu.add)
                    dview = OUT.rearrange("(h p) r -> p h r",
                                          p=128)[:, :, csl_of(c)]
                    nc.sync.dma_start(dview, ot[:])

                # Vector: xreg for g-1 (critical), then trailing stages
                if 1 <= g <= NG:
                    c = g - 1
                    for k in range(2):
                        nc.vector.tensor_tensor(xreg[:, k, csl_of(c)],
                                                xt[:, k, csl_of(c)],
                                                rpb[:], Alu.mult)

                # PE: main matmuls for chunk g-1; the xreg-independent
                # A/identity parts are issued before the B parts so they
                # overlap the broadcast -> xreg chain.
                if 1 <= g <= NG:
                    c = g - 1
                    csl = csl_of(c)
                    pm = pmpool.tile([128, 2, G], f32, tag="pm",
                                     name=f"pm{c}")
                    pm_t[c] = pm
                    for nb in range(2):
                        nc.tensor.matmul(pm[:, nb, :],
                                         wm[:, 0, 128 * nb:128 * (nb + 1)],
                                         xt[:, 0, csl], start=True,
                                         stop=False)
                        nc.tensor.matmul(pm[:, nb, :],
                                         wm[:, 1, 128 * nb:128 * (nb + 1)],
                                         xt[:, 1, csl], start=False,
                                         stop=False)
                        nc.tensor.matmul(pm[:, nb, :],
                                         wm[:, 0, 256 + 128 * nb:
                                            256 + 128 * (nb + 1)],
                                         xreg[:, 0, csl], start=False,
                                         stop=False)
                        nc.tensor.matmul(pm[:, nb, :],
                                         wm[:, 1, 256 + 128 * nb:
                                            256 + 128 * (nb + 1)],
                                         xreg[:, 1, csl], start=False,
                                         stop=False)
                        nc.tensor.matmul(pm[:, nb, :], i128[:],
                                         xb3t[:, nb, csl], start=False,
                                         stop=True)

                # ACT: CNN ladder for group g (must not sit behind evac)
                if g < NG:
                    eh1 = cnnpool.tile([128, G], f32, tag="eh1",
                                       name=f"eh1g{g}")
                    nc.scalar.activation(eh1[:], p1[:], AF.Exp, bias=b2v[:])
                    h1s = cnnpool.tile([128, G], f16, tag="h1s",
                                       name=f"h1sg{g}")
                    nc.scalar.activation(h1s[:], eh1[:], AF.Ln, bias=1.0)
                    p2 = pc23pool.tile([32, G], f32, tag="p23",
                                       name=f"p2g{g}")
                    nc.tensor.matmul(p2[:], m2bd[:], h1s[:],
                                     start=True, stop=True)
                    eh2 = cnnpool.tile([32, G], f32, tag="eh2",
                                       name=f"eh2g{g}")
                    nc.scalar.activation(eh2[:], p2[:], AF.Exp, bias=b3v[:])
                    h2s = cnnpool.tile([32, G], f16, tag="h2s",
                                       name=f"h2sg{g}")
                    nc.scalar.activation(h2s[:], eh2[:], AF.Ln, bias=1.0)
                    p3 = pc23pool.tile([2, G], f32, tag="p23",
                                       name=f"p3g{g}")
                    nc.tensor.matmul(p3[:], lwbd[:], h2s[:],
                                     start=True, stop=True)
                    spE = cnnpool.tile([2, G], f32, tag="spE",
                                       name=f"spEg{g}")
                    nc.scalar.activation(spE[:], p3[:], AF.Exp,
                                         bias=lbmr[:])
                    nc.scalar.activation(sp[:, gsl], spE[:], AF.Ln,
                                         bias=1.0)
                    # mu row is on partition 1 — broadcast sources must
                    # start at partition 0; hop via DMA
                    nc.sync.dma_start(murow[:, gsl], sp[1:2, gsl])

                # ACT: s evacuation PSUM -> SBUF f16 for chunk g-1
                if 1 <= g <= NG:
                    c = g - 1
                    s16 = s16pool.tile([128, 2 * G], f16, tag="s16",
                                       name=f"s16c{c}")
                    s16_t[c] = s16
                    nc.scalar.activation(
                        s16[:], pm_t[c][:].rearrange("p a b -> p (a b)"),
                        AF.Copy)

                # Vector: hm/q for chunk g-2 (from evacuated s)
                if 2 <= g <= NG + 1:
                    c = g - 2
                    csl = csl_of(c)
                    hm = hqpool.tile([128, 2, G], f16, tag="hm",
                                     name=f"hm{c}")
                    q = hqpool.tile([128, 2, G], f16, tag="q", name=f"q{c}")
                    hm_t[c] = hm
                    q_t[c] = q
                    for nb in range(2):
                        nsl = slice(G * nb, G * (nb + 1))
                        nc.vector._custom_dve(
                            COPS['HM'], out=hm[:, nb, :],
                            in0=s16_t[c][:, nsl], in1=mub[:, csl],
                            s0=gm2 * 2.0 / 3.0, s1=1.0 / 12.0, imm2=0.5)
                        nc.vector._custom_dve(
                            COPS['QQ'], out=q[:, nb, :],
                            in0=s16_t[c][:, nsl], in1=mub[:, csl],
                            s0=gm2, s1=-0.25, imm2=0.5)

                # ACT: ihm3 = 1/(3 hm) via exp(-ln(3 hm)) for chunk g-2
                if 2 <= g <= NG + 1:
                    c = g - 2
                    lh = midpool.tile([128, 2 * G], f16, tag="lh",
                                      name=f"lh{c}")
                    nc.scalar.activation(
                        lh[:], hm_t[c][:].rearrange("p a b -> p (a b)"),
                        AF.Ln, scale=3.0)
                    ihm3 = midpool.tile([128, 2 * G], f16, tag="ihm3",
                                        name=f"ihm3{c}")
                    nc.scalar.activation(ihm3[:], lh[:], AF.Exp, scale=-1.0)
                    ihm3_t[c] = ihm3

                # Vector: sigma chain for chunk g-3
                if 3 <= g <= NG + 2:
                    c = g - 3
                    qf = q_t[c][:].rearrange("p a b -> p (a b)")
                    e = midpool.tile([128, 2 * G], f16, tag="e",
                                     name=f"e{c}")
                    nc.vector._custom_dve(
                        COPS['SIGE'], out=e[:], in0=qf, in1=ihm3_t[c][:],
                        s0=SIG_C0, s1=SIG_C1, imm2=SIG_C2)
                    H = midpool.tile([128, 2 * G], f16, tag="H",
                                     name=f"H{c}")
                    H_t[c] = H
                    nc.vector._custom_dve(COPS['QE3'], out=H[:], in0=qf,
                                          in1=e[:], s0=1.0)
                    rg = midpool.tile([128, 2 * G], f16, tag="rg",
                                      name=f"rg{c}")
                    rg_t[c] = rg
                    nc.vector.tensor_tensor(rg[:], H[:], ihm3_t[c][:],
                                            Alu.mult)


    nc.compile()
    return nc


def _get_program(gamma, m):
    key = (B_TOTAL, N, N_CORES, float(gamma), float(m))
    if key not in _PROG:
        _PROG[key] = _build_program(gamma, m)
    return _PROG[key]


def _host_prep(inputs):
    f16 = np.float16
    x = _np_f32(inputs['x']).reshape(B_TOTAL, N)
    x_b = _np_f32(inputs['x_b']).reshape(B_TOTAL, N)
    m = float(np.asarray(inputs['mass']).reshape(-1)[0])
    gp = float(np.asarray(inputs['gamma_p']).reshape(-1)[0])
    gamma = float(np.log1p(np.exp(gp))) if gp < 30 else gp
    TtT = _np_f32(inputs['TtT'])
    DtD = _np_f32(inputs['DtD'])

    W_A = ((np.eye(N, dtype=np.float32) - np.float32(gamma) * TtT.T)
           / np.float32(3.0 * m))
    W_B = -np.float32(gamma) * DtD.T / np.float32(3.0 * m)
    WM = np.ascontiguousarray(
        np.concatenate([W_A, W_B], axis=1).astype(f16))        # (256,512)

    M1s, M2s, lws = {}, {}, {}
    for tag in ('mu', 'reg'):
        M1s[tag] = _conv_pool_mat(inputs['w2_' + tag], 256)
        M2s[tag] = _conv_pool_mat(inputs['w3_' + tag], 64)
        lws[tag] = _np_f32(inputs['lw_' + tag]).reshape(16)
    M1cat = np.concatenate([M1s['mu'], M1s['reg']], axis=0)     # (128,256)
    M1T = np.ascontiguousarray(M1cat.T.astype(f16))             # (256,128)
    M2BD = np.zeros((128, 32), f16)
    M2BD[0:64, 0:16] = M2s['mu'].T.astype(f16)
    M2BD[64:128, 16:32] = M2s['reg'].T.astype(f16)
    # column 0 -> reg (broadcast straight from partition 0), column 1 -> mu
    LWBD = np.zeros((32, 2), f16)
    LWBD[16:32, 0] = lws['reg'].astype(f16)
    LWBD[0:16, 1] = lws['mu'].astype(f16)

    def sc(name):
        return float(np.asarray(inputs[name]).reshape(-1)[0])

    B2V = np.full((128, 1), sc('b2_mu'), np.float32)
    B2V[64:] = sc('b2_reg')
    B3V = np.full((32, 1), sc('b3_mu'), np.float32)
    B3V[16:] = sc('b3_reg')
    LBMR = np.array([[sc('lb_reg')], [sc('lb_mu')]], np.float32)
    I128 = np.eye(128, dtype=f16)

    consts = dict(wm=WM, i128=I128, m1t=M1T, m2bd=M2BD, lwbd=LWBD,
                  b2v=B2V, b3v=B3V, lbmr=LBMR)

    xb3 = (np.float32(gamma / (3.0 * m)) * x_b
           + np.float32(1.0 / 3.0)).astype(np.float32)
    in_maps = []
    for c in range(N_CORES):
        rows = slice(BC * c, BC * (c + 1))
        im = dict(consts)
        im['xt'] = np.ascontiguousarray(x[rows].T.astype(f16))
        im['xb3t'] = np.ascontiguousarray(xb3[rows].T.astype(f16))
        in_maps.append(im)
    return in_maps, m, gamma


def kernel(**inputs) -> np.ndarray:
    from concourse import bass_utils
    in_maps, m, gamma = _host_prep(inputs)
    nc = _get_program(gamma, m)
    res = bass_utils.run_bass_kernel_spmd(nc, in_maps,
                                          core_ids=list(range(N_CORES)))
    parts = [res.results[c]['out'].T.astype(np.float32)
             for c in range(N_CORES)]
    out = np.concatenate(parts, axis=0)
    if m != 1.0:
        out = (np.float32(m) * out).astype(np.float32)
    return np.ascontiguousarray(out.reshape(B_TOTAL, 1, N))
